# revision 1
# baseline (speedup 1.0000x reference)
"""TRN2 Bass kernel for nn_BlendEmoBackbone: gated audio mixer + low-rank
multiplicative fusion, data-parallel over batch on 8 NeuronCores.

Strategy:
- Pure data parallel: each core handles B/8 = 512 batch rows; the gate MLP
  weights and the LMF factor tensors are replicated.
- All activations kept in transposed [feature, batch] layout on-chip so every
  matmul contracts over the partition dim; tokens are transposed on the host.
- LayerNorms over the feature dim use PE ones-matmuls for partition sums,
  with the -mu term folded into gate matmuls as an extra contraction row.
- The LMF where(mask, z, 1) and the x_aug ones-column are folded into the
  factor matmul as a K=2 tail tile ([bias_row; ones_row] x [mask; 1-mask]).
- rank_w is folded into the audio factor slices on the host.
- Matmuls run in float32r (TF32-like, full PE rate for N>=256).
"""

import numpy as np
from contextlib import ExitStack

import concourse.bass as bass
from concourse import bacc
import concourse.tile as tile
from concourse import mybir
from concourse.bass_utils import run_bass_kernel_spmd

B, M, H, R = 4096, 4, 1024, 10
NCORES = 8
BS = B // NCORES          # 512 batch rows per core
MID = 512
P = 128
HT = H // P               # 8 h-tiles
MT = MID // P             # 4 mid-tiles
D3 = 3 * H
OTHERS = (0, 2, 3)
AUDIO = 1
EPS = 1e-5

f32 = mybir.dt.float32
f32r = mybir.dt.float32r
u8 = mybir.dt.uint8
AF = mybir.ActivationFunctionType
OP = mybir.AluOpType

TRACE = False
LAST_RESULTS = None

_cached_nc = None


def _build():
    nc = bacc.Bacc("TRN2", target_bir_lowering=False, debug=False)

    # ---- DRAM parameters (per core) ----
    tokT = nc.declare_dram_parameter("tokT", [M, H, BS], f32r, isOutput=False)
    # u8 rows: 0-2 pv_j, 3-5 mo_j, 6 am(aum), 7 ma
    u8rows = nc.declare_dram_parameter("u8rows", [8, BS], u8, isOutput=False)
    cmrows = nc.declare_dram_parameter("cmrows", [3, BS], f32, isOutput=False)
    uv = nc.declare_dram_parameter("uv", [M, 2, BS], f32r, isOutput=False)
    WGO = nc.declare_dram_parameter("WGO", [3 * HT, P, MID], f32r, isOutput=False)
    WGA = nc.declare_dram_parameter("WGA", [3 * HT, P, MID], f32r, isOutput=False)
    WGOe = nc.declare_dram_parameter("WGOe", [1, MID], f32r, isOutput=False)
    WGAe = nc.declare_dram_parameter("WGAe", [1, MID], f32r, isOutput=False)
    W2 = nc.declare_dram_parameter("W2", [P, MT, 2], f32r, isOutput=False)
    CB = nc.declare_dram_parameter("CB", [P, 8], f32, isOutput=False)
    SC = nc.declare_dram_parameter("SC", [1, 8], f32, isOutput=False)
    # [ht_out, kt, P, P] tiled weight blocks (lhsT layout)
    A2OT = nc.declare_dram_parameter("A2OT", [HT, HT, P, P], f32r, isOutput=False)
    O2AT = nc.declare_dram_parameter("O2AT", [HT, HT, P, P], f32r, isOutput=False)
    OUTWT = nc.declare_dram_parameter("OUTWT", [HT, HT, P, P], f32r, isOutput=False)
    # cols: ln_o_w 0:8, ln_o_b 8:16, ln_a_w 16:24, ln_a_b 24:32,
    #       ln1w 32:40, ln1b 40:48, ln2w 48:56, ln2b 56:64, outb 64:72, lmfb 72:80
    LNV = nc.declare_dram_parameter("LNV", [P, 80], f32, isOutput=False)
    # FT[..., 8, 0:2, :] = [bias_row; ones_or_rankw_row], rest of block 8 unused
    FT = nc.declare_dram_parameter("FT", [R, HT, M, 9, P, P], f32r, isOutput=False)
    KON = nc.declare_dram_parameter("KON", [P, 1], f32r, isOutput=False)
    OUT = nc.declare_dram_parameter("outT", [H, BS], f32, isOutput=True)

    with tile.TileContext(nc) as tc, ExitStack() as ctx:
        kp = ctx.enter_context(tc.tile_pool(name="konst", bufs=1))
        tokp = ctx.enter_context(tc.tile_pool(name="tokp", bufs=1))
        big = ctx.enter_context(tc.tile_pool(name="big", bufs=1))
        wk = ctx.enter_context(tc.tile_pool(name="wk", bufs=2))
        bcp = ctx.enter_context(tc.tile_pool(name="bcp", bufs=1))
        sqp = ctx.enter_context(tc.tile_pool(name="sqp", bufs=2))
        wgp = ctx.enter_context(tc.tile_pool(name="wgp", bufs=2))
        ftp = ctx.enter_context(tc.tile_pool(name="ftp", bufs=2))
        rowp = ctx.enter_context(tc.tile_pool(name="rowp", bufs=1))
        ppz = ctx.enter_context(tc.tile_pool(name="ppz", bufs=4, space="PSUM"))
        pps = ctx.enter_context(tc.tile_pool(name="pps", bufs=1, space="PSUM"))
        ppo = ctx.enter_context(tc.tile_pool(name="ppo", bufs=2, space="PSUM"))

        # ---- constants / small loads ----
        ones_k = kp.tile([P, 1], f32r)
        nc.sync.dma_start(out=ones_k, in_=KON.ap())
        ones1 = kp.tile([1, P], f32)
        nc.vector.memset(ones1, 1.0)

        def bc_row_dma(dst, src_ap):
            nc.sync.dma_start(
                out=dst,
                in_=bass.AP(
                    tensor=src_ap.tensor, offset=src_ap.offset, ap=[[0, P], [1, BS]]
                ),
            )

        u8t = []
        for i in range(8):
            t = kp.tile([P, BS], u8, tag=f"u8_{i}")
            bc_row_dma(t, u8rows.ap()[i : i + 1, :])
            u8t.append(t)
        pv_t, mo_t, am_t, ma_t = u8t[0:3], u8t[3:6], u8t[6], u8t[7]
        cm_t = []
        for i in range(3):
            t = kp.tile([P, BS], f32, tag=f"cm_{i}")
            bc_row_dma(t, cmrows.ap()[i : i + 1, :])
            cm_t.append(t)
        uvt = []
        for m in range(M):
            t = kp.tile([2, BS], f32r, tag=f"uv_{m}")
            nc.sync.dma_start(out=t, in_=uv.ap()[m])
            uvt.append(t)
        cbt = kp.tile([P, 8], f32)
        nc.sync.dma_start(out=cbt, in_=CB.ap())
        sct = kp.tile([1, 8], f32)
        nc.sync.dma_start(out=sct, in_=SC.ap())
        lnv = kp.tile([P, 80], f32)
        nc.sync.dma_start(out=lnv, in_=LNV.ap())
        w2t = kp.tile([P, MT, 2], f32r)
        nc.sync.dma_start(out=w2t, in_=W2.ap())

        # ---- tokens (transposed) ----
        tok = tokp.tile([P, M, HT, BS], f32r)
        for m in range(M):
            src = tokT.ap()[m].rearrange("(ht p) b -> p ht b", p=P)
            for ht in range(HT):
                nc.sync.dma_start(out=tok[:, m, ht, :], in_=src[:, ht, :])

        def tk(m, kt):
            return tok[:, m, kt, :]

        # ---- helpers ----
        def ln_rows(stat, n, tag):
            """From psum stat banks (sum, sumsq) compute negmu [1,BS] f32r
            and rinv [1,BS] f32 rows."""
            statA, statB = stat
            negmu = rowp.tile([1, BS], f32r, tag="negmu", name=f"negmu_{tag}")
            nc.scalar.activation(negmu, statA[0:1, :], AF.Copy, bias=0.0, scale=-1.0 / n)
            ex2 = rowp.tile([1, BS], f32, tag="ex2", name=f"ex2_{tag}")
            nc.scalar.activation(ex2, statB[0:1, :], AF.Copy, bias=0.0, scale=1.0 / n)
            msq = rowp.tile([1, BS], f32, tag="msq", name=f"msq_{tag}")
            nc.scalar.activation(msq, negmu, AF.Square)
            nc.vector.tensor_sub(ex2, ex2, msq)                      # var in place
            nc.scalar.activation(msq, ex2, AF.Sqrt, bias=sct[0:1, 2:3], scale=1.0)  # sd
            rinv = rowp.tile([1, BS], f32, tag="rinv", name=f"rinv_{tag}")
            nc.vector.reciprocal(rinv, msq)
            return negmu, rinv

        def bcast(row, tag):
            """Broadcast a [1,BS] f32 row to a [P,BS] f32 sbuf tile via PE outer."""
            po = ppo.tile([P, BS], f32, tag="outer")
            nc.tensor.matmul(po, ones1, row, start=True, stop=True)
            sb = bcp.tile([P, BS], f32, tag=f"bc_{tag}")
            nc.vector.tensor_copy(sb, po)
            return sb

        def colsum_stats(stat, pairs):
            """Accumulate sum (bank A) and sumsq (bank B) over the given
            (tile, square_tile) pairs of [P,BS] f32r tiles."""
            statA, statB = stat
            n = len(pairs)
            for i, (t, sq) in enumerate(pairs):
                nc.tensor.matmul(statA[0:1, :], ones_k, t, start=(i == 0), stop=(i == n - 1))
                nc.tensor.matmul(statB[0:1, :], ones_k, sq, start=(i == 0), stop=(i == n - 1))

        # ---- a2o = audio @ a2o_w.T, in T layout [H, BS] ----
        a2or = big.tile([P, HT, BS], f32, tag="axr")
        for ho in range(HT):
            ps = ppz.tile([P, BS], f32, tag="z")
            for kt in range(HT):
                wt = wgp.tile([P, P], f32r, tag="ww")
                nc.sync.dma_start(out=wt, in_=A2OT.ap()[ho, kt])
                nc.tensor.matmul(ps, wt, tk(AUDIO, kt), start=(kt == 0), stop=(kt == HT - 1))
            nc.vector.tensor_copy(a2or[:, ho, :], ps)

        omt = big.tile([P, HT, BS], f32r, tag="om")  # others_mean accumulator
        mix_src = {"x": a2or}  # a2o for others-gates, o2a for the audio gate

        def gate_and_mix(j, mj):
            """j: 0..2 index into OTHERS, or 3 for the audio gate."""
            is_audio = j == 3
            t_m = AUDIO if is_audio else mj

            def s_tile(kt):
                return omt[:, kt, :] if is_audio else tk(AUDIO, kt)

            # |t-s| tiles + squares + LN stats over the 3H concat features
            stat = (pps.tile([1, BS], f32, tag="statA", name="statA"),
            pps.tile([1, BS], f32, tag="statB", name="statB"))
            abs_t = big.tile([P, HT, BS], f32r, tag="abs")
            pairs = []
            for kt in range(HT):
                d = wk.tile([P, BS], f32, tag="d")
                nc.vector.tensor_sub(d, tk(t_m, kt), s_tile(kt))
                nc.scalar.activation(abs_t[:, kt, :], d, AF.Abs)
                sqd = sqp.tile([P, BS], f32r, tag="sq")
                nc.vector.tensor_mul(sqd, d, d)
                pairs.append((abs_t[:, kt, :], sqd))
                sqt = sqp.tile([P, BS], f32r, tag="sq")
                nc.vector.tensor_mul(sqt, tk(t_m, kt), tk(t_m, kt))
                pairs.append((tk(t_m, kt), sqt))
                sqs = sqp.tile([P, BS], f32r, tag="sq")
                nc.vector.tensor_mul(sqs, s_tile(kt), s_tile(kt))
                pairs.append((s_tile(kt), sqs))
            colsum_stats(stat, pairs)
            negmu, rinv = ln_rows(stat, D3, "g")

            # gate layer 1: psum[mt] = W~.T @ [t; s; |t-s|] - mu*c1
            WG = WGA if is_audio else WGO
            wge = rowp.tile([1, MID], f32r, tag="wge", name=f"wge{j}")
            nc.sync.dma_start(out=wge, in_=(WGAe if is_audio else WGOe).ap())
            gps = [ppz.tile([P, BS], f32, tag="z", name=f"gps{mt}") for mt in range(MT)]
            for kt in range(3 * HT):
                wt = wgp.tile([P, MID], f32r, tag="wg")
                nc.sync.dma_start(out=wt, in_=WG.ap()[kt])
                part, k = kt // HT, kt % HT
                rhs = tk(t_m, k) if part == 0 else (s_tile(k) if part == 1 else abs_t[:, k, :])
                for mt in range(MT):
                    nc.tensor.matmul(
                        gps[mt], wt[:, mt * P : (mt + 1) * P], rhs,
                        start=(kt == 0), stop=False,
                    )
            for mt in range(MT):
                nc.tensor.matmul(
                    gps[mt], wge[0:1, mt * P : (mt + 1) * P], negmu,
                    start=False, stop=True,
                )
            rb = bcast(rinv, "rb")
            cb_off = 4 if is_audio else 0
            col = 1 if is_audio else 0
            gp = pps.tile([1, BS], f32, tag="statA", name="gp")
            for mt in range(MT):
                hm = wk.tile([P, BS], f32, tag="hm")
                nc.vector.tensor_mul(hm, gps[mt], rb)
                hg1 = wk.tile([P, BS], f32r, tag="hg", name=f"hg{mt}")
                nc.scalar.activation(
                    hg1, hm, AF.Gelu,
                    bias=cbt[:, cb_off + mt : cb_off + mt + 1], scale=1.0,
                )
                nc.tensor.matmul(
                    gp[:, :], w2t[:, mt, col : col + 1], hg1,
                    start=(mt == 0), stop=(mt == MT - 1),
                )
            g_row = rowp.tile([1, BS], f32, tag="g_row")
            nc.scalar.activation(
                g_row, gp[:, :], AF.Sigmoid,
                bias=sct[0:1, col : col + 1], scale=1.0,
            )
            gb = bcast(g_row, "gb")

            # pre = t + g * (a2o | o2a); LN over H; blend into tok in place
            src = mix_src["x"]
            pre = big.tile([P, HT, BS], f32r, tag="abs", name="pre")
            stat2 = (pps.tile([1, BS], f32, tag="statA", name="stat2A"),
            pps.tile([1, BS], f32, tag="statB", name="stat2B"))
            pairs2 = []
            for kt in range(HT):
                tmp = wk.tile([P, BS], f32, tag="hm")
                nc.vector.tensor_mul(tmp, gb, src[:, kt, :])
                nc.vector.tensor_add(pre[:, kt, :], tmp, tk(t_m, kt))
                sq = sqp.tile([P, BS], f32r, tag="sq")
                nc.vector.tensor_mul(sq, pre[:, kt, :], pre[:, kt, :])
                pairs2.append((pre[:, kt, :], sq))
            colsum_stats(stat2, pairs2)
            negmu2, rinv2 = ln_rows(stat2, H, "u")
            mb = bcast(negmu2.bitcast(f32), "mb")
            rb2 = bcast(rinv2, "rb2")
            wcol = 16 if is_audio else 0
            bcol = 24 if is_audio else 8
            for kt in range(HT):
                nc.vector.tensor_add(pre[:, kt, :], pre[:, kt, :], mb)
                nc.vector.tensor_mul(pre[:, kt, :], pre[:, kt, :], rb2)
                nc.vector.tensor_scalar(
                    pre[:, kt, :], pre[:, kt, :],
                    lnv[:, wcol + kt : wcol + kt + 1], lnv[:, bcol + kt : bcol + kt + 1],
                    op0=OP.mult, op1=OP.add,
                )
                # blend = big_mask*t + small_mask*(upd - t), in place into tok
                bm = ma_t if is_audio else mo_t[j]
                sm = am_t if is_audio else pv_t[j]
                d2 = wk.tile([P, BS], f32, tag="d", name="d2")
                nc.vector.tensor_sub(d2, pre[:, kt, :], tk(t_m, kt))
                nc.vector.tensor_mul(d2, d2, sm)
                nc.vector.tensor_mul(tk(t_m, kt), tk(t_m, kt), bm)
                nc.vector.tensor_add(tk(t_m, kt), tk(t_m, kt), d2)
                if not is_audio:
                    if j == 0:
                        nc.vector.tensor_mul(omt[:, kt, :], cm_t[j], tk(mj, kt))
                    else:
                        tmp2 = wk.tile([P, BS], f32, tag="hm")
                        nc.vector.tensor_mul(tmp2, cm_t[j], tk(mj, kt))
                        nc.vector.tensor_add(omt[:, kt, :], omt[:, kt, :], tmp2)

        for j, mj in enumerate(OTHERS):
            gate_and_mix(j, mj)

        # ---- o2a = others_mean @ o2a_w.T ----
        o2ar = big.tile([P, HT, BS], f32, tag="axr")
        for ho in range(HT):
            ps = ppz.tile([P, BS], f32, tag="z")
            for kt in range(HT):
                wt = wgp.tile([P, P], f32r, tag="ww")
                nc.sync.dma_start(out=wt, in_=O2AT.ap()[ho, kt])
                nc.tensor.matmul(ps, wt, omt[:, kt, :], start=(kt == 0), stop=(kt == HT - 1))
            nc.vector.tensor_copy(o2ar[:, ho, :], ps)
        mix_src["x"] = o2ar

        gate_and_mix(3, AUDIO)

        # ---- LMF ----
        acc = big.tile([P, HT, BS], f32r, tag="acc")
        for r in range(R):
            for ht in range(HT):
                zps = []
                for m in range(M):
                    ft = ftp.tile([P, 9, P], f32r, tag="ft")
                    nc.sync.dma_start(
                        out=ft, in_=FT.ap()[r, ht, m].rearrange("kt p c -> p kt c")
                    )
                    zp = ppz.tile([P, BS], f32, tag="z")
                    for kt in range(HT):
                        nc.tensor.matmul(
                            zp, ft[:, kt, :], tk(m, kt), start=(kt == 0), stop=False
                        )
                    nc.tensor.matmul(zp, ft[0:2, 8, :], uvt[m], start=False, stop=True)
                    zps.append(zp)
                s0 = wk.tile([P, BS], f32, tag="s0")
                nc.vector.tensor_copy(s0, zps[0])
                nc.vector.tensor_mul(s0, s0, zps[1])
                nc.vector.tensor_mul(s0, s0, zps[2])
                if r == 0:
                    nc.vector.tensor_mul(acc[:, ht, :], s0, zps[3])
                else:
                    nc.vector.tensor_mul(s0, s0, zps[3])
                    nc.vector.tensor_add(acc[:, ht, :], acc[:, ht, :], s0)

        # ---- output MLP ----
        stat3 = (pps.tile([1, BS], f32, tag="statA", name="stat3A"),
            pps.tile([1, BS], f32, tag="statB", name="stat3B"))
        pairs3 = []
        for kt in range(HT):
            nc.vector.tensor_scalar_add(
                acc[:, kt, :], acc[:, kt, :], lnv[:, 72 + kt : 72 + kt + 1]
            )
            sq = sqp.tile([P, BS], f32r, tag="sq")
            nc.vector.tensor_mul(sq, acc[:, kt, :], acc[:, kt, :])
            pairs3.append((acc[:, kt, :], sq))
        colsum_stats(stat3, pairs3)
        negmu3, rinv3 = ln_rows(stat3, H, "l1")
        mb3 = bcast(negmu3.bitcast(f32), "mb")
        rb3 = bcast(rinv3, "rb2")
        for kt in range(HT):
            nc.vector.tensor_add(acc[:, kt, :], acc[:, kt, :], mb3)
            nc.vector.tensor_mul(acc[:, kt, :], acc[:, kt, :], rb3)
            nc.vector.tensor_scalar(
                acc[:, kt, :], acc[:, kt, :],
                lnv[:, 32 + kt : 32 + kt + 1], lnv[:, 40 + kt : 40 + kt + 1],
                op0=OP.mult, op1=OP.add,
            )

        # h2 = gelu(h1 @ out_w.T + out_b); LN2; write out
        h2 = big.tile([P, HT, BS], f32r, tag="abs")
        stat4 = (pps.tile([1, BS], f32, tag="statA", name="stat4A"),
            pps.tile([1, BS], f32, tag="statB", name="stat4B"))
        pairs4 = []
        for ho in range(HT):
            ps = ppz.tile([P, BS], f32, tag="z")
            for kt in range(HT):
                wt = wgp.tile([P, P], f32r, tag="ww")
                nc.sync.dma_start(out=wt, in_=OUTWT.ap()[ho, kt])
                nc.tensor.matmul(ps, wt, acc[:, kt, :], start=(kt == 0), stop=(kt == HT - 1))
            nc.scalar.activation(
                h2[:, ho, :], ps, AF.Gelu, bias=lnv[:, 64 + ho : 64 + ho + 1], scale=1.0
            )
            sq = sqp.tile([P, BS], f32r, tag="sq")
            nc.vector.tensor_mul(sq, h2[:, ho, :], h2[:, ho, :])
            pairs4.append((h2[:, ho, :], sq))
        colsum_stats(stat4, pairs4)
        negmu4, rinv4 = ln_rows(stat4, H, "l2")
        mb4 = bcast(negmu4.bitcast(f32), "mb")
        rb4 = bcast(rinv4, "rb2")
        for kt in range(HT):
            fin = wk.tile([P, BS], f32, tag="fin")
            nc.vector.tensor_add(fin, h2[:, kt, :], mb4)
            nc.vector.tensor_mul(fin, fin, rb4)
            nc.vector.tensor_scalar(
                fin, fin, lnv[:, 48 + kt : 48 + kt + 1], lnv[:, 56 + kt : 56 + kt + 1],
                op0=OP.mult, op1=OP.add,
            )
            nc.sync.dma_start(out=OUT.ap()[kt * P : (kt + 1) * P, :], in_=fin)

    nc.compile()
    return nc


def _host_prep(inputs):
    tokens = np.asarray(inputs["tokens"], np.float32)
    token_mask = np.asarray(inputs["token_mask"])
    mask_f = token_mask.astype(np.float32)

    mo = mask_f[:, list(OTHERS)]                      # [B,3]
    ma = mask_f[:, AUDIO]                             # [B]
    pv = mo * ma[:, None]                             # [B,3]
    winv = (1.0 / np.clip(mo.sum(1), 1.0, None)).astype(np.float32)
    aum = ma * (mo.max(1) > 0)                        # [B]

    go_w1 = np.asarray(inputs["go_w1"], np.float32)
    ga_w1 = np.asarray(inputs["ga_w1"], np.float32)

    def gate_prep(w1, b1, lnw, lnb):
        W1w = w1 * lnw[None, :]                       # [MID, 3H]
        c1 = np.ascontiguousarray(W1w.sum(1).reshape(1, MID))
        cb = w1 @ lnb + b1                            # [MID]
        Wblocks = np.ascontiguousarray(W1w.T).reshape(3 * HT, P, MID)
        return Wblocks, c1, cb

    WGOv, c1go, cbgo = gate_prep(
        go_w1, np.asarray(inputs["go_b1"], np.float32),
        np.asarray(inputs["ln_go_w"], np.float32), np.asarray(inputs["ln_go_b"], np.float32),
    )
    WGAv, c1ga, cbga = gate_prep(
        ga_w1, np.asarray(inputs["ga_b1"], np.float32),
        np.asarray(inputs["ln_ga_w"], np.float32), np.asarray(inputs["ln_ga_b"], np.float32),
    )
    CBv = np.ascontiguousarray(
        np.concatenate([cbgo.reshape(MT, P).T, cbga.reshape(MT, P).T], axis=1)
    ).astype(np.float32)                              # [P, 8]
    W2v = np.stack(
        [np.asarray(inputs["go_w2"], np.float32).reshape(MID),
         np.asarray(inputs["ga_w2"], np.float32).reshape(MID)], axis=1
    )                                                 # [MID, 2]
    W2v = np.ascontiguousarray(W2v.reshape(MT, P, 2).transpose(1, 0, 2))
    SCv = np.zeros((1, 8), np.float32)
    SCv[0, 0] = np.asarray(inputs["go_b2"], np.float32).reshape(-1)[0]
    SCv[0, 1] = np.asarray(inputs["ga_b2"], np.float32).reshape(-1)[0]
    SCv[0, 2] = EPS

    def tile_blocks(w):
        wt = np.ascontiguousarray(np.asarray(w, np.float32).T)    # [H_in, H_out]
        return np.ascontiguousarray(wt.reshape(HT, P, HT, P).transpose(2, 0, 1, 3))

    A2OTv = tile_blocks(inputs["a2o_w"])
    O2ATv = tile_blocks(inputs["o2a_w"])
    OUTWTv = tile_blocks(inputs["out_w"])

    def cols(name):
        return np.asarray(inputs[name], np.float32).reshape(HT, P).T

    LNVv = np.zeros((P, 80), np.float32)
    for i, name in enumerate(
        ["ln_o_w", "ln_o_b", "ln_a_w", "ln_a_b", "out_ln1_w", "out_ln1_b",
         "out_ln2_w", "out_ln2_b", "out_b", "lmf_bias"]
    ):
        LNVv[:, 8 * i : 8 * (i + 1)] = cols(name)

    factors = np.asarray(inputs["factors"], np.float32)
    rank_w = np.asarray(inputs["rank_w"], np.float32)
    Ff = factors.copy()
    Ff[AUDIO] = Ff[AUDIO] * rank_w[:, None, None]
    FTv = np.zeros((R, HT, M, 9, P, P), np.float32)
    main = Ff[:, :, 1:, :].reshape(M, R, HT, P, HT, P)   # [m, r, kt, pk, ht, ph]
    FTv[:, :, :, :8, :, :] = main.transpose(1, 4, 0, 2, 3, 5)
    bias = Ff[:, :, 0, :].reshape(M, R, HT, P)           # [m, r, ht, ph]
    FTv[:, :, :, 8, 0, :] = bias.transpose(1, 2, 0, 3)
    ones_row = np.ones((R, HT, M, P), np.float32)
    ones_row[:, :, AUDIO, :] = rank_w[:, None, None]
    FTv[:, :, :, 8, 1, :] = ones_row

    shared = dict(
        WGO=WGOv, WGA=WGAv, WGOe=c1go, WGAe=c1ga, W2=W2v, CB=CBv, SC=SCv,
        A2OT=A2OTv, O2AT=O2ATv, OUTWT=OUTWTv, LNV=LNVv, FT=FTv,
        KON=np.ones((P, 1), np.float32),
    )

    in_maps = []
    for c in range(NCORES):
        sl = slice(c * BS, (c + 1) * BS)
        tokTv = np.ascontiguousarray(tokens[sl].transpose(1, 2, 0))  # [M, H, BS]
        u8v = np.zeros((8, BS), np.uint8)
        u8v[0:3] = pv[sl].T > 0
        u8v[3:6] = mo[sl].T > 0
        u8v[6] = aum[sl] > 0
        u8v[7] = ma[sl] > 0
        cmv = np.ascontiguousarray((mo[sl] * winv[sl, None]).T.astype(np.float32))
        uvv = np.zeros((M, 2, BS), np.float32)
        uvv[:, 0, :] = mask_f[sl].T
        uvv[:, 1, :] = 1.0 - mask_f[sl].T
        in_maps.append(dict(tokT=tokTv, u8rows=u8v, cmrows=cmv, uv=uvv, **shared))
    return in_maps


def kernel(**inputs):
    global _cached_nc, LAST_RESULTS
    if _cached_nc is None:
        _cached_nc = _build()
    in_maps = _host_prep(inputs)
    res = run_bass_kernel_spmd(
        _cached_nc, in_maps, core_ids=list(range(NCORES)), trace=TRACE
    )
    LAST_RESULTS = res
    out = np.stack([res.results[c]["outT"].T for c in range(NCORES)], axis=0)
    return np.ascontiguousarray(out.reshape(B, H)).astype(np.float32)



# revision 4
# speedup vs baseline: 1.8319x; 1.8319x over previous
"""TRN2 Bass kernel v2 for nn_BlendEmoBackbone.

Strategy vs v1 baseline (2.24ms):
- bf16 matmuls/activations everywhere (fp32r HIGH ran at ~427ns/512-row MM;
  bf16 should halve that). Verified numerically: full-bf16 LMF path is 5.2e-3
  rel err vs the 2e-2 gate.
- Batch columns globally sorted by mask pattern and re-dealt across the 8
  cores so every core has the identical group-size layout (compiled per mask;
  cached). Gates/mixer then run on narrow contiguous spans, and the audio LMF
  factor matmuls stream only the ma=1 half. Correctness never depends on the
  grouping: true-mask tiles (pv/am/uv) gate every blend, so a row placed in a
  superset group just does discarded extra work.
- Host premasks tokens (tokens*mask), so masked columns contribute exactly 0
  to every matmul; the K=2 uv-tail writes bias into active cols and 1.0 into
  inactive cols of each z-psum.
- Engine rebalance: squares/abs/gelu on Scalar, LMF z-products on GpSimd,
  everything else elementwise on DVE (bf16 2x mode).
- FT factors stored [R,HT,P,M,8,P] bf16: one contiguous 1MB DMA per (r,ht),
  8KB per-partition lines.
"""

import numpy as np
from contextlib import ExitStack

import concourse.bass as bass
from concourse import bacc
import concourse.tile as tile
from concourse import mybir
from concourse.bass_utils import run_bass_kernel_spmd

B, M, H, R = 4096, 4, 1024, 10
NCORES = 8
BSP = 512                 # batch columns per core (exact, via group flow)
MID = 512
P = 128
HT = H // P
MT = MID // P
D3 = 3 * H
AUDIO = 1
OTHERS = (0, 2, 3)        # token index of gate j = OTHERS[j]
# pattern bit of token m: pat = 8*ma + 4*m0 + 2*m2 + 1*m3
BIT = {1: 8, 0: 4, 2: 2, 3: 1}
EPS = 1e-5
NG = 16

f32 = mybir.dt.float32
bf16 = mybir.dt.bfloat16
AF = mybir.ActivationFunctionType
OP = mybir.AluOpType

TRACE = False
LAST_RESULTS = None

_cache = {}


# ---------------------------------------------------------------- layout --
def _compute_layout(token_mask):
    mask = np.asarray(token_mask).astype(bool)
    ma = mask[:, 1]
    pat = 8 * ma + 4 * mask[:, 0] + 2 * mask[:, 2] + 1 * mask[:, 3]
    n = np.bincount(pat, minlength=NG)

    s = np.ceil(n / 8).astype(int)
    excess = int(s.sum()) - BSP // 1
    # reduce group capacities until sum(s)==512, preferring pure slack, then
    # low-popcount groups (their surplus rows flow up into superset groups)
    order = sorted(range(NG), key=lambda g: (bin(g).count("1"), g))
    while excess > 0:
        done = False
        for g in order:
            if s[g] > 0 and 8 * (s[g] - 1) >= n[g]:
                s[g] -= 1
                excess -= 1
                done = True
                break
        if done:
            continue
        for g in order:
            if g == NG - 1:
                continue
            if s[g] > 0:
                s[g] -= 1
                excess -= 1
                done = True
                break
        assert done, "cannot reduce capacities"
    assert s.sum() == BSP

    # flow rows into groups: pattern g rows -> group g, overflow to smallest
    # superset with free capacity. Most-constrained (high popcount) first.
    cap = 8 * s
    members = [[] for _ in range(NG)]
    pats_by_constraint = sorted(range(NG), key=lambda g: (-bin(g).count("1"), -g))
    rows_of = [np.nonzero(pat == g)[0] for g in range(NG)]
    for g in pats_by_constraint:
        rows = list(rows_of[g])
        take = min(len(rows), cap[g] - len(members[g]))
        members[g].extend(rows[:take])
        rest = rows[take:]
        if rest:
            sups = sorted(
                (g2 for g2 in range(NG) if (g2 & g) == g and g2 != g),
                key=lambda g2: bin(g2).count("1"),
            )
            for g2 in sups:
                room = cap[g2] - len(members[g2])
                if room > 0:
                    members[g2].extend(rest[:room])
                    rest = rest[room:]
                if not rest:
                    break
            assert not rest, f"group flow infeasible for pattern {g}"
    for g in range(NG):
        assert len(members[g]) == cap[g]

    # deal: core c takes slice [c*s_g:(c+1)*s_g] of each group
    cols = np.empty((NCORES, BSP), np.int64)
    for c in range(NCORES):
        parts = [
            np.asarray(members[g][c * s[g] : (c + 1) * s[g]], np.int64)
            for g in range(NG)
        ]
        cols[c] = np.concatenate(parts)

    off = np.zeros(NG + 1, int)
    off[1:] = np.cumsum(s)
    return dict(key=tuple(int(x) for x in s), s=s, off=off, cols=cols)


# ----------------------------------------------------------------- build --
def _build(layout):
    off = layout["off"]
    off8 = int(off[8])    # audio / upper half starts here
    off9 = int(off[9])    # aum / a2o / o2a / omean span
    off10 = int(off[10])  # gate m2 span
    off12 = int(off[12])  # gate m0 span
    W8 = BSP - off8
    W9 = BSP - off9
    # gate spans: j=0 -> m0, j=1 -> m2, j=2 -> m3, j=3 -> audio
    GOFF = [off12, off10, off9, off9]

    nc = bacc.Bacc("TRN2", target_bir_lowering=False, debug=False)

    TOK = nc.declare_dram_parameter("TOK", [P, M, HT, BSP], bf16, isOutput=False)
    MR = nc.declare_dram_parameter("MR", [4, BSP], bf16, isOutput=False)  # pv0,pv2,pv3,am
    DINV = nc.declare_dram_parameter("DINV", [1, BSP], f32, isOutput=False)
    UV = nc.declare_dram_parameter("UV", [M, 2, BSP], bf16, isOutput=False)
    WGO = nc.declare_dram_parameter("WGO", [3 * HT, P, MID], bf16, isOutput=False)
    WGA = nc.declare_dram_parameter("WGA", [3 * HT, P, MID], bf16, isOutput=False)
    WGOe = nc.declare_dram_parameter("WGOe", [1, MID], bf16, isOutput=False)
    WGAe = nc.declare_dram_parameter("WGAe", [1, MID], bf16, isOutput=False)
    W2 = nc.declare_dram_parameter("W2", [P, MT, 2], bf16, isOutput=False)
    CB = nc.declare_dram_parameter("CB", [P, 8], f32, isOutput=False)
    SC = nc.declare_dram_parameter("SC", [1, 8], f32, isOutput=False)
    A2OT = nc.declare_dram_parameter("A2OT", [HT, HT, P, P], bf16, isOutput=False)
    O2AT = nc.declare_dram_parameter("O2AT", [HT, HT, P, P], bf16, isOutput=False)
    OUTWT = nc.declare_dram_parameter("OUTWT", [HT, HT, P, P], bf16, isOutput=False)
    # cols: ln_o_w 0:8, ln_o_b 8:16, ln_a_w 16:24, ln_a_b 24:32,
    #       ln1w 32:40, ln1b 40:48, ln2w 48:56, ln2b 56:64, outb 64:72, lmfb 72:80
    LNV = nc.declare_dram_parameter("LNV", [P, 80], f32, isOutput=False)
    FT = nc.declare_dram_parameter("FT", [R, HT, P, M, 8, P], bf16, isOutput=False)
    FTT = nc.declare_dram_parameter("FTT", [R, HT, 2, M, P], bf16, isOutput=False)
    KON = nc.declare_dram_parameter("KON", [P, 1], bf16, isOutput=False)
    OUT = nc.declare_dram_parameter("outT", [H, BSP], f32, isOutput=True)

    with tile.TileContext(nc) as tc, ExitStack() as ctx:
        kp = ctx.enter_context(tc.tile_pool(name="konst", bufs=1))
        tokp = ctx.enter_context(tc.tile_pool(name="tokp", bufs=1))
        big = ctx.enter_context(tc.tile_pool(name="big", bufs=1))
        wk = ctx.enter_context(tc.tile_pool(name="wk", bufs=2))
        bcp = ctx.enter_context(tc.tile_pool(name="bcp", bufs=1))
        wgp = ctx.enter_context(tc.tile_pool(name="wgp", bufs=2))
        ftp = ctx.enter_context(tc.tile_pool(name="ftp", bufs=3))
        ft2 = ctx.enter_context(tc.tile_pool(name="ft2", bufs=3))
        rowp = ctx.enter_context(tc.tile_pool(name="rowp", bufs=1))
        prp = ctx.enter_context(tc.tile_pool(name="prp", bufs=1))
        ppz = ctx.enter_context(tc.tile_pool(name="ppz", bufs=6, space="PSUM"))
        ppo = ctx.enter_context(tc.tile_pool(name="ppo", bufs=1, space="PSUM"))
        pps = ctx.enter_context(tc.tile_pool(name="pps", bufs=1, space="PSUM"))

        # ---- constants / small loads ----
        ones_k = kp.tile([P, 1], bf16)
        nc.sync.dma_start(out=ones_k, in_=KON.ap())
        ones1b = kp.tile([1, P], bf16)
        nc.vector.memset(ones1b, 1.0)
        ones1f = kp.tile([1, P], f32)
        nc.vector.memset(ones1f, 1.0)

        def bc_row_dma(dst, src_ap, w):
            nc.sync.dma_start(
                out=dst,
                in_=bass.AP(
                    tensor=src_ap.tensor, offset=src_ap.offset, ap=[[0, P], [1, w]]
                ),
            )

        # pv masks broadcast to [P, span]; am likewise
        pvb = []
        for j in range(3):
            w = BSP - GOFF[j]
            t = kp.tile([P, w], bf16, tag=f"pv_{j}")
            bc_row_dma(t, MR.ap()[j : j + 1, GOFF[j] :], w)
            pvb.append(t)
        amb = kp.tile([P, W9], bf16, tag="amb")
        bc_row_dma(amb, MR.ap()[3:4, off9:], W9)

        dinvr = kp.tile([1, BSP], f32)
        nc.sync.dma_start(out=dinvr, in_=DINV.ap())
        uvt = []
        for m in range(M):
            t = kp.tile([2, BSP], bf16, tag=f"uv_{m}")
            nc.sync.dma_start(out=t, in_=UV.ap()[m])
            uvt.append(t)
        cbt = kp.tile([P, 8], f32)
        nc.sync.dma_start(out=cbt, in_=CB.ap())
        sct = kp.tile([1, 8], f32)
        nc.sync.dma_start(out=sct, in_=SC.ap())
        lnv = kp.tile([P, 80], f32)
        nc.sync.dma_start(out=lnv, in_=LNV.ap())
        w2t = kp.tile([P, MT, 2], bf16)
        nc.sync.dma_start(out=w2t, in_=W2.ap())

        # ---- tokens: one contiguous DMA ----
        tok = tokp.tile([P, M, HT, BSP], bf16)
        nc.sync.dma_start(out=tok, in_=TOK.ap())

        def tk(m, kt):
            return tok[:, m, kt, :]

        # ---- helpers ----
        def ln_rows(stat, n, w, tag):
            """negmu (bf16) and rinv (f32) rows [1,w] from psum stats."""
            statA, statB = stat
            negmu = rowp.tile([1, w], bf16, tag="negmu", name=f"negmu_{tag}")
            nc.scalar.activation(negmu, statA, AF.Copy, bias=0.0, scale=-1.0 / n)
            ex2 = rowp.tile([1, w], f32, tag="ex2", name=f"ex2_{tag}")
            nc.scalar.activation(ex2, statB, AF.Copy, bias=0.0, scale=1.0 / n)
            msq = rowp.tile([1, w], f32, tag="msq", name=f"msq_{tag}")
            nc.scalar.activation(msq, negmu, AF.Square)
            nc.vector.tensor_sub(ex2, ex2, msq)  # var in place
            nc.scalar.activation(msq, ex2, AF.Sqrt, bias=sct[0:1, 2:3], scale=1.0)
            rinv = rowp.tile([1, w], f32, tag="rinv", name=f"rinv_{tag}")
            nc.vector.reciprocal(rinv, msq)
            return negmu, rinv

        def bcast(row, w, tag, out_dt=bf16):
            """[1,w] row -> [P,w] tile via PE outer; copy out through Scalar."""
            ones = ones1b if row.dtype == bf16 else ones1f
            po = ppo.tile([P, BSP], f32, tag="outer")
            nc.tensor.matmul(po[:, :w], ones, row, start=True, stop=True)
            sb = bcp.tile([P, w], out_dt, tag=f"bc_{tag}")
            nc.scalar.activation(sb, po[:, :w], AF.Copy)
            return sb

        def colsum_stats(w, pairs, tag):
            """sum (row 0) and sumsq (row 32) over partition dim via PE."""
            st = pps.tile([33, BSP], f32, tag="stat", name=f"stat_{tag}")
            statA, statB = st[0:1, :w], st[32:33, :w]
            npair = len(pairs)
            for i, (t, sq) in enumerate(pairs):
                nc.tensor.matmul(statA, ones_k, t, start=(i == 0), stop=(i == npair - 1))
                nc.tensor.matmul(statB, ones_k, sq, start=(i == 0), stop=(i == npair - 1))
            return statA, statB

        # ---- audio squares on span9 (shared by the 3 others-gates) ----
        asq = big.tile([P, HT, W9], bf16, tag="asq")
        for kt in range(HT):
            nc.scalar.activation(asq[:, kt, :], tok[:, AUDIO, kt, off9:], AF.Square)

        # ---- a2o = audio @ a2o_w.T on span9 ----
        a2or = big.tile([P, HT, W9], bf16, tag="axr")
        for ho in range(HT):
            ps = ppz.tile([P, BSP], f32, tag="z")
            for kt in range(HT):
                wt = wgp.tile([P, P], bf16, tag="ww")
                nc.sync.dma_start(out=wt, in_=A2OT.ap()[ho, kt])
                nc.tensor.matmul(
                    ps[:, :W9], wt, tok[:, AUDIO, kt, off9:],
                    start=(kt == 0), stop=(kt == HT - 1),
                )
            nc.scalar.activation(a2or[:, ho, :], ps[:, :W9], AF.Copy)

        omean = big.tile([P, HT, W9], bf16, tag="om")

        def gate_and_mix(j, src):
            """j: 0..2 others-gates (token OTHERS[j]), 3 audio gate."""
            is_audio = j == 3
            t_m = AUDIO if is_audio else OTHERS[j]
            go = GOFF[j]
            w = BSP - go
            o9 = go - off9  # offset of this gate's span inside span9 tiles

            def t_tile(kt):
                return tok[:, t_m, kt, go:]

            def s_tile(kt):
                return omean[:, kt, o9:] if is_audio else tok[:, AUDIO, kt, go:]

            # |t-s| + squares + LN stats over 3H concat features
            abs_t = big.tile([P, HT, w], bf16, tag="abs", name=f"abs{j}")
            sq_t = prp.tile([P, HT, w], bf16, tag="sqt", name=f"sqt{j}")
            sq_d = prp.tile([P, HT, w], bf16, tag="sqd", name=f"sqd{j}")
            sq_s = (
                prp.tile([P, HT, w], bf16, tag="sqs", name=f"sqs{j}")
                if is_audio else None
            )
            pairs = []
            for kt in range(HT):
                d = wk.tile([P, w], bf16, tag="d")
                nc.vector.tensor_sub(d, t_tile(kt), s_tile(kt))
                nc.scalar.activation(abs_t[:, kt, :], d, AF.Abs)
                nc.scalar.activation(sq_d[:, kt, :], d, AF.Square)
                nc.scalar.activation(sq_t[:, kt, :], t_tile(kt), AF.Square)
                if is_audio:
                    nc.scalar.activation(sq_s[:, kt, :], s_tile(kt), AF.Square)
                    ss = sq_s[:, kt, :]
                else:
                    ss = asq[:, kt, o9:]
                pairs.append((t_tile(kt), sq_t[:, kt, :]))
                pairs.append((s_tile(kt), ss))
                pairs.append((abs_t[:, kt, :], sq_d[:, kt, :]))
            statA, statB = colsum_stats(w, pairs, f"g{j}")
            negmu, rinv = ln_rows((statA, statB), D3, w, f"g{j}")

            # gate layer 1
            WG = WGA if is_audio else WGO
            wge = rowp.tile([1, MID], bf16, tag="wge", name=f"wge{j}")
            nc.sync.dma_start(out=wge, in_=(WGAe if is_audio else WGOe).ap())
            gps = [
                ppz.tile([P, BSP], f32, tag="z", name=f"gps{j}_{mt}")
                for mt in range(MT)
            ]
            for kt in range(3 * HT):
                wt = wgp.tile([P, MID], bf16, tag="wg")
                nc.sync.dma_start(out=wt, in_=WG.ap()[kt])
                part, k = kt // HT, kt % HT
                rhs = (
                    t_tile(k) if part == 0
                    else (s_tile(k) if part == 1 else abs_t[:, k, :])
                )
                for mt in range(MT):
                    nc.tensor.matmul(
                        gps[mt][:, :w], wt[:, mt * P : (mt + 1) * P], rhs,
                        start=(kt == 0), stop=False,
                    )
            for mt in range(MT):
                nc.tensor.matmul(
                    gps[mt][:, :w], wge[0:1, mt * P : (mt + 1) * P], negmu,
                    start=False, stop=True,
                )
            rb = bcast(rinv, w, "rb")
            cb_off = 4 if is_audio else 0
            col = 1 if is_audio else 0
            gp = pps.tile([33, BSP], f32, tag="stat", name=f"gp{j}")
            for mt in range(MT):
                hm = wk.tile([P, w], bf16, tag="hm")
                nc.vector.tensor_mul(hm, gps[mt][:, :w], rb)
                hg1 = wk.tile([P, w], bf16, tag="hg", name=f"hg{mt}")
                nc.scalar.activation(
                    hg1, hm, AF.Gelu,
                    bias=cbt[:, cb_off + mt : cb_off + mt + 1], scale=1.0,
                )
                nc.tensor.matmul(
                    gp[0:1, :w], w2t[:, mt, col : col + 1], hg1,
                    start=(mt == 0), stop=(mt == MT - 1),
                )
            g_row = rowp.tile([1, w], f32, tag="g_row")
            nc.scalar.activation(
                g_row, gp[0:1, :w], AF.Sigmoid,
                bias=sct[0:1, col : col + 1], scale=1.0,
            )
            gb = bcast(g_row, w, "gb")

            # pre = t + g*(a2o|o2a); LN over H; masked blend into tok
            pre = big.tile([P, HT, w], bf16, tag="abs", name=f"pre{j}")
            sqp = prp.tile([P, HT, w], bf16, tag="sqt", name=f"sqp{j}")
            pairs2 = []
            for kt in range(HT):
                tmp = wk.tile([P, w], bf16, tag="hm2")
                nc.vector.tensor_mul(tmp, gb, src[:, kt, o9:])
                nc.vector.tensor_add(pre[:, kt, :], tmp, t_tile(kt))
                nc.scalar.activation(sqp[:, kt, :], pre[:, kt, :], AF.Square)
                pairs2.append((pre[:, kt, :], sqp[:, kt, :]))
            statA2, statB2 = colsum_stats(w, pairs2, f"u{j}")
            negmu2, rinv2 = ln_rows((statA2, statB2), H, w, f"u{j}")
            mb = bcast(negmu2, w, "mb")
            rb2 = bcast(rinv2, w, "rb2")
            wcol = 16 if is_audio else 0
            bcol = 24 if is_audio else 8
            msk = amb[:, o9:] if is_audio else pvb[j]
            for kt in range(HT):
                nc.vector.tensor_add(pre[:, kt, :], pre[:, kt, :], mb)
                nc.vector.tensor_mul(pre[:, kt, :], pre[:, kt, :], rb2)
                nc.vector.tensor_scalar(
                    pre[:, kt, :], pre[:, kt, :],
                    lnv[:, wcol + kt : wcol + kt + 1],
                    lnv[:, bcol + kt : bcol + kt + 1],
                    op0=OP.mult, op1=OP.add,
                )
                d2 = wk.tile([P, w], bf16, tag="d", name="d2")
                nc.vector.tensor_sub(d2, pre[:, kt, :], t_tile(kt))
                nc.vector.tensor_mul(d2, d2, msk)
                nc.vector.tensor_add(t_tile(kt), t_tile(kt), d2)

        for j in range(3):
            gate_and_mix(j, a2or)

        # ---- others_mean on span9 ----
        dvb = bcast(dinvr[0:1, off9:], W9, "dv")
        for kt in range(HT):
            osum = wk.tile([P, W9], bf16, tag="hm2", name=f"osum{kt}")
            nc.vector.tensor_add(
                osum, tok[:, OTHERS[0], kt, off9:], tok[:, OTHERS[1], kt, off9:]
            )
            nc.vector.tensor_add(osum, osum, tok[:, OTHERS[2], kt, off9:])
            nc.vector.tensor_mul(omean[:, kt, :], osum, dvb)

        # ---- o2a = others_mean @ o2a_w.T on span9 ----
        o2ar = big.tile([P, HT, W9], bf16, tag="axr", name="o2ar")
        for ho in range(HT):
            ps = ppz.tile([P, BSP], f32, tag="z")
            for kt in range(HT):
                wt = wgp.tile([P, P], bf16, tag="ww")
                nc.sync.dma_start(out=wt, in_=O2AT.ap()[ho, kt])
                nc.tensor.matmul(
                    ps[:, :W9], wt, omean[:, kt, :],
                    start=(kt == 0), stop=(kt == HT - 1),
                )
            nc.scalar.activation(o2ar[:, ho, :], ps[:, :W9], AF.Copy)

        gate_and_mix(3, o2ar)

        # ---- LMF ----
        # audio matmuls stream only [off8:]; other m full width. The uv tail
        # (start=True, full width) zeroes/fills: bias*mask + 1*(1-mask).
        acc = big.tile([P, HT, BSP], f32, tag="acc")
        covers = [[(0, BSP)], [(off8, BSP)], [(0, BSP)], [(0, BSP)]]
        for r in range(R):
            for ht in range(HT):
                ft = ftp.tile([P, M, 8, P], bf16, tag="ft")
                nc.sync.dma_start(out=ft, in_=FT.ap()[r, ht])
                ftt = ft2.tile([2, M, P], bf16, tag="ftt")
                nc.sync.dma_start(out=ftt, in_=FTT.ap()[r, ht])
                zps = []
                for m in range(M):
                    zp = ppz.tile([P, BSP], f32, tag="z")
                    nc.tensor.matmul(
                        zp, ftt[:, m, :], uvt[m],
                        start=True, stop=False, skip_group_check=True,
                    )
                    cov = covers[m]
                    for kt in range(8):
                        for ci, (a, b) in enumerate(cov):
                            nc.tensor.matmul(
                                zp[:, a:b], ft[:, m, kt, :], tok[:, m, kt, a:b],
                                start=False,
                                stop=(kt == 7 and ci == len(cov) - 1),
                                skip_group_check=True,
                            )
                    zps.append(zp)
                zb0 = wk.tile([P, BSP], bf16, tag="zb0")
                nc.scalar.activation(zb0, zps[0], AF.Copy)
                zb2 = wk.tile([P, BSP], bf16, tag="zb2")
                nc.scalar.activation(zb2, zps[2], AF.Copy)
                s01 = wk.tile([P, BSP], bf16, tag="s01")
                nc.vector.tensor_mul(s01, zb0, zps[1])
                s23 = wk.tile([P, BSP], bf16, tag="s23")
                nc.vector.tensor_mul(s23, zb2, zps[3])
                if r == 0:
                    nc.gpsimd.tensor_mul(acc[:, ht, :], s01, s23)
                else:
                    prod = wk.tile([P, BSP], bf16, tag="prod")
                    nc.gpsimd.tensor_mul(prod, s01, s23)
                    nc.gpsimd.tensor_add(acc[:, ht, :], acc[:, ht, :], prod)

        # ---- output MLP ----
        hb = big.tile([P, HT, BSP], bf16, tag="abs", name="hb")
        sqh = prp.tile([P, HT, BSP], bf16, tag="sqt", name="sqh")
        pairs3 = []
        for kt in range(HT):
            nc.scalar.activation(
                hb[:, kt, :], acc[:, kt, :], AF.Identity,
                bias=lnv[:, 72 + kt : 72 + kt + 1], scale=1.0,
            )
            nc.scalar.activation(sqh[:, kt, :], hb[:, kt, :], AF.Square)
            pairs3.append((hb[:, kt, :], sqh[:, kt, :]))
        statA3, statB3 = colsum_stats(BSP, pairs3, "l1")
        negmu3, rinv3 = ln_rows((statA3, statB3), H, BSP, "l1")
        mb3 = bcast(negmu3, BSP, "mb")
        rb3 = bcast(rinv3, BSP, "rb2")
        for kt in range(HT):
            nc.vector.tensor_add(hb[:, kt, :], hb[:, kt, :], mb3)
            nc.vector.tensor_mul(hb[:, kt, :], hb[:, kt, :], rb3)
            nc.vector.tensor_scalar(
                hb[:, kt, :], hb[:, kt, :],
                lnv[:, 32 + kt : 32 + kt + 1], lnv[:, 40 + kt : 40 + kt + 1],
                op0=OP.mult, op1=OP.add,
            )

        # h2 = gelu(h1 @ out_w.T + out_b); LN2; write out
        h2 = big.tile([P, HT, BSP], bf16, tag="h2")
        sq2 = prp.tile([P, HT, BSP], bf16, tag="sqd", name="sq2")
        pairs4 = []
        for ho in range(HT):
            ps = ppz.tile([P, BSP], f32, tag="z")
            for kt in range(HT):
                wt = wgp.tile([P, P], bf16, tag="ww")
                nc.sync.dma_start(out=wt, in_=OUTWT.ap()[ho, kt])
                nc.tensor.matmul(ps, wt, hb[:, kt, :], start=(kt == 0), stop=(kt == HT - 1))
            nc.scalar.activation(
                h2[:, ho, :], ps, AF.Gelu, bias=lnv[:, 64 + ho : 64 + ho + 1], scale=1.0
            )
            nc.scalar.activation(sq2[:, ho, :], h2[:, ho, :], AF.Square)
            pairs4.append((h2[:, ho, :], sq2[:, ho, :]))
        statA4, statB4 = colsum_stats(BSP, pairs4, "l2")
        negmu4, rinv4 = ln_rows((statA4, statB4), H, BSP, "l2")
        mb4 = bcast(negmu4, BSP, "mb")
        rb4 = bcast(rinv4, BSP, "rb2")
        for kt in range(HT):
            fin = wk.tile([P, BSP], f32, tag="fin")
            nc.vector.tensor_add(fin, h2[:, kt, :], mb4)
            nc.vector.tensor_mul(fin, fin, rb4)
            nc.vector.tensor_scalar(
                fin, fin, lnv[:, 48 + kt : 48 + kt + 1], lnv[:, 56 + kt : 56 + kt + 1],
                op0=OP.mult, op1=OP.add,
            )
            nc.sync.dma_start(out=OUT.ap()[kt * P : (kt + 1) * P, :], in_=fin)

    nc.compile()
    return nc


# ------------------------------------------------------------- host prep --
def _host_prep(inputs, layout):
    import ml_dtypes

    tokens = np.asarray(inputs["tokens"], np.float32)
    token_mask = np.asarray(inputs["token_mask"]).astype(bool)
    mask_f = token_mask.astype(np.float32)
    tok_m = tokens * mask_f[:, :, None]

    ma = mask_f[:, AUDIO]
    mo = mask_f[:, list(OTHERS)]
    aum = ma * (mo.max(1) > 0)
    dinv_all = (1.0 / np.clip(mo.sum(1), 1.0, None)).astype(np.float32)

    def gate_prep(w1, b1, lnw, lnb):
        W1w = w1 * lnw[None, :]
        c1 = np.ascontiguousarray(W1w.sum(1).reshape(1, MID))
        cb = w1 @ lnb + b1
        Wblocks = np.ascontiguousarray(W1w.T).reshape(3 * HT, P, MID)
        return Wblocks, c1, cb

    bfc = lambda a: np.ascontiguousarray(a).astype(ml_dtypes.bfloat16)

    WGOv, c1go, cbgo = gate_prep(
        np.asarray(inputs["go_w1"], np.float32), np.asarray(inputs["go_b1"], np.float32),
        np.asarray(inputs["ln_go_w"], np.float32), np.asarray(inputs["ln_go_b"], np.float32),
    )
    WGAv, c1ga, cbga = gate_prep(
        np.asarray(inputs["ga_w1"], np.float32), np.asarray(inputs["ga_b1"], np.float32),
        np.asarray(inputs["ln_ga_w"], np.float32), np.asarray(inputs["ln_ga_b"], np.float32),
    )
    CBv = np.ascontiguousarray(
        np.concatenate([cbgo.reshape(MT, P).T, cbga.reshape(MT, P).T], axis=1)
    ).astype(np.float32)
    W2v = np.stack(
        [np.asarray(inputs["go_w2"], np.float32).reshape(MID),
         np.asarray(inputs["ga_w2"], np.float32).reshape(MID)], axis=1
    )
    W2v = np.ascontiguousarray(W2v.reshape(MT, P, 2).transpose(1, 0, 2))
    SCv = np.zeros((1, 8), np.float32)
    SCv[0, 0] = np.asarray(inputs["go_b2"], np.float32).reshape(-1)[0]
    SCv[0, 1] = np.asarray(inputs["ga_b2"], np.float32).reshape(-1)[0]
    SCv[0, 2] = EPS

    def tile_blocks(w):
        wt = np.ascontiguousarray(np.asarray(w, np.float32).T)
        return bfc(wt.reshape(HT, P, HT, P).transpose(2, 0, 1, 3))

    A2OTv = tile_blocks(inputs["a2o_w"])
    O2ATv = tile_blocks(inputs["o2a_w"])
    OUTWTv = tile_blocks(inputs["out_w"])

    def cols_(name):
        return np.asarray(inputs[name], np.float32).reshape(HT, P).T

    LNVv = np.zeros((P, 80), np.float32)
    for i, name in enumerate(
        ["ln_o_w", "ln_o_b", "ln_a_w", "ln_a_b", "out_ln1_w", "out_ln1_b",
         "out_ln2_w", "out_ln2_b", "out_b", "lmf_bias"]
    ):
        LNVv[:, 8 * i : 8 * (i + 1)] = cols_(name)

    factors = np.asarray(inputs["factors"], np.float32)
    rank_w = np.asarray(inputs["rank_w"], np.float32)
    Ff = factors.copy()
    Ff[AUDIO] = Ff[AUDIO] * rank_w[:, None, None]
    # FT[r, ht, pk, m, kt, ph] = Ff[m, r, kt*128+pk, ht*128+ph]
    Fm = Ff[:, :, 1:, :].reshape(M, R, 8, P, HT, P)
    FTv = bfc(Fm.transpose(1, 4, 3, 0, 2, 5))
    FTTv = np.zeros((2, R, HT, M, P), np.float32)
    FTTv[0] = Ff[:, :, 0, :].reshape(M, R, HT, P).transpose(1, 2, 0, 3)
    FTTv[1] = 1.0
    FTTv[1, :, :, AUDIO, :] = rank_w[:, None, None]
    FTTv = bfc(FTTv.transpose(1, 2, 0, 3, 4))

    shared = dict(
        WGO=bfc(WGOv), WGA=bfc(WGAv), WGOe=bfc(c1go), WGAe=bfc(c1ga),
        W2=bfc(W2v), CB=CBv, SC=SCv, A2OT=A2OTv, O2AT=O2ATv, OUTWT=OUTWTv,
        LNV=LNVv, FT=FTv, FTT=FTTv, KON=bfc(np.ones((P, 1), np.float32)),
    )

    pv = (mask_f[:, list(OTHERS)] * ma[:, None]) > 0  # [B,3]

    in_maps = []
    for c in range(NCORES):
        cix = layout["cols"][c]
        ta = tok_m[cix]  # [BSP, M, H]
        TOKv = bfc(ta.reshape(BSP, M, HT, P).transpose(3, 1, 2, 0))
        MRv = np.zeros((4, BSP), np.float32)
        MRv[0] = pv[cix, 0]
        MRv[1] = pv[cix, 1]
        MRv[2] = pv[cix, 2]
        MRv[3] = aum[cix]
        DINVv = np.ascontiguousarray(dinv_all[cix].reshape(1, BSP))
        UVv = np.zeros((M, 2, BSP), np.float32)
        UVv[:, 0, :] = mask_f[cix].T
        UVv[:, 1, :] = 1.0 - mask_f[cix].T
        in_maps.append(
            dict(TOK=TOKv, MR=bfc(MRv), DINV=DINVv, UV=bfc(UVv), **shared)
        )
    return in_maps


def kernel(**inputs):
    global LAST_RESULTS
    layout = _compute_layout(inputs["token_mask"])
    key = layout["key"]
    if key not in _cache:
        _cache[key] = _build(layout)
    nc = _cache[key]
    in_maps = _host_prep(inputs, layout)
    res = run_bass_kernel_spmd(
        nc, in_maps, core_ids=list(range(NCORES)), trace=TRACE
    )
    LAST_RESULTS = res
    out = np.zeros((B, H), np.float32)
    for c in range(NCORES):
        out[layout["cols"][c]] = res.results[c]["outT"].T
    return out
